# revision 10
# baseline (speedup 1.0000x reference)
"""Trainium2 Bass kernel for nn_Detector (retrieval_knn drift detector).

Math (per token):
    z  = encoder(x + noise) = relu((x+n) @ W1 + b1) @ W2 + b2
    cls = argmin_j ||z - c_j||     (reference uses encoder(x); noise is 1e-2
                                    so computing the argmin on the noisy
                                    encoding leaves the drift bit unchanged)
    d2 = ||z - c_cls||^2
    drift = (d2 > A_cls) | (d2 < B_cls),  A=(med+3.5 mad)^2, B=(med-3.5 mad)^2

Host algebra (c' = c - b2, z' = bias-free encoding):
    argmin_j ||z - c_j|| == argmax_j M_j,  M_j = 256*(z'.c'_j + q_j),
        q_j = S - 0.5||c'_j||^2
    m1 = max_j M_j  =>  256*d2 = 256*||z'||^2 - 2*m1 + 512*S  (c'* cancels)
    onehot = (M >= m1)  -- DVE tensor_scalar at 4x -- is DMA-transposed
    (hw xbar) to [j-part, tok] layout, then a tiny PE matmul with the
    constant table [E_j | D_j] selects both thresholds exactly:
        E_j = 256*(2S - A_j),  D_j = 256*B_j - 512*S
        f1 = (w + E* > 0),  f2 = (w < D*),  w = 256*||z'||^2 - 2*m1
    drift = f1 | f2.

All big matmuls fp8e4 DoubleRow (K=256 packed / 128 partitions, 0.5
cycles/row); q rides in contraction rows 128/129 (hi+lo fp8 split).
Scales: x*16, W1*64, h*32, W2*32, z*16, c*16 => scores/d2 in 256*units.

The select matmuls + drift flags for tile i are emitted during tile i+1
(one-stage software pipeline) so the PE never stalls on the
copy->tree->onehot->transpose latency chain.

Pure data-parallel over 8 NeuronCores (8192 tokens each).
"""

import numpy as np
import ml_dtypes

import concourse.bass as bass
import concourse.bacc as bacc
import concourse.mybir as mybir
import concourse.tile as tile
from concourse.masks import make_identity

FP8 = ml_dtypes.float8_e4m3
BF16 = ml_dtypes.bfloat16

B, D_IN, H, D_LAT, K = 65536, 512, 256, 128, 1000
MAD_THRESHOLD = 3.5
N_CORES = 8
BS = B // N_CORES            # tokens per core
TOK_TILE = 512               # tokens per pipeline tile
CH = TOK_TILE // 128         # 4 token chunks per tile
KP = 1024                    # padded centroid count (xbar needs /128)
PRE_S = 64.0                 # q shift

SC_X = 16.0
SC_W1 = 64.0
SC_H = 32.0
SC_W2 = 32.0
SC_Z = 16.0

DR = mybir.MatmulPerfMode.DoubleRow


def build_program(n_tiles=BS // TOK_TILE, b1_zero=True):
    bs = n_tiles * TOK_TILE
    nc = bacc.Bacc(
        "TRN2",
        target_bir_lowering=False,
        debug=False,
        enable_asserts=False,
        num_devices=N_CORES,
    )
    f32, bf16, fp8, i32 = (
        mybir.dt.float32, mybir.dt.bfloat16, mybir.dt.float8e4, mybir.dt.int32,
    )

    xnT = nc.dram_tensor("xnT", [n_tiles, 128, 2, 2, TOK_TILE], fp8,
                         kind="ExternalInput").ap()
    W1d = nc.dram_tensor("W1d", [128, 2, 2, H], fp8, kind="ExternalInput").ap()
    W2d = nc.dram_tensor("W2d", [128, 2, D_LAT], fp8,
                         kind="ExternalInput").ap()
    cMd = nc.dram_tensor("cMd", [128, 2, K], fp8, kind="ExternalInput").ap()
    EDd = nc.dram_tensor("EDd", [128, 8, 2], bf16, kind="ExternalInput").ap()
    b1d = nc.dram_tensor("b1d", [128, 2], f32, kind="ExternalInput").ap()
    drift_d = nc.dram_tensor("drift", [bs], i32, kind="ExternalOutput").ap()

    with tile.TileContext(nc) as tc:
        with (
            tc.tile_pool(name="const", bufs=1) as const,
            tc.tile_pool(name="xin", bufs=3) as xin,
            tc.tile_pool(name="hsb", bufs=2) as hsb,
            tc.tile_pool(name="z2tp", bufs=2) as z2tp,
            tc.tile_pool(name="tree", bufs=3) as treep,
            tc.tile_pool(name="ohp", bufs=3) as ohp,
            tc.tile_pool(name="ohtp", bufs=8) as ohtp,
            tc.tile_pool(name="junk", bufs=3) as junkp,
            tc.tile_pool(name="ta", bufs=6) as tap,
            tc.tile_pool(name="flags", bufs=10) as small,
            tc.tile_pool(name="acc", bufs=1) as accp,
            tc.tile_pool(name="mm", bufs=1, space="PSUM") as mmp,
            tc.tile_pool(name="ztp", bufs=1, space="PSUM") as ztpp,
            tc.tile_pool(name="selp", bufs=1, space="PSUM") as selpp,
            tc.tile_pool(name="gp", bufs=2, space="PSUM") as gpp,
        ):
            # ---- constants -------------------------------------------------
            W1s = const.tile([128, 2, 2, H], fp8)
            nc.sync.dma_start(W1s[:], W1d[:])
            W2s = const.tile([128, 2, D_LAT], fp8)
            nc.sync.dma_start(W2s[:], W2d[:])
            cMs = const.tile([128, 2, K], fp8)
            nc.sync.dma_start(cMs[:], cMd[:])
            EDs = const.tile([128, 8, 2], bf16)
            nc.sync.dma_start(EDs[:], EDd[:])
            b1s = const.tile([128, 2], f32)
            nc.sync.dma_start(b1s[:], b1d[:])
            ident = const.tile([128, 128], f32)
            make_identity(nc, ident[:])

            msbufs = []
            for k in range(3):
                mb = const.tile([128, KP], bf16, tag=f"mb{k}")
                nc.gpsimd.memset(mb[:, K:KP], 0.0)
                msbufs.append(mb)

            z2bufs = []
            for k in range(2):
                zb = const.tile([128, 2, TOK_TILE], fp8, tag=f"zb{k}")
                nc.gpsimd.memset(zb[:, 1, :], 0.0)
                nc.gpsimd.memset(zb[0:2, 1, :], SC_Z)
                z2bufs.append(zb)

            driftacc = accp.tile([128, n_tiles * CH], f32)

            halves = [(0, 512), (512, K)]

            def emit_sel_flags(st):
                """Select matmuls + drift flags for a finished tile."""
                ip, ohTs, m8s, zsqT = st
                sel = selpp.tile([128, CH, 2], f32, tag="sel")
                for c in range(CH):
                    for g in range(8):
                        nc.tensor.matmul(
                            sel[:, c, :], lhsT=ohTs[c][:, g, :],
                            rhs=EDs[:, g, :],
                            start=(g == 0), stop=(g == 7),
                        )
                wT = small.tile([128, CH], f32, tag="w")
                nc.vector.scalar_tensor_tensor(
                    out=wT[:], in0=m8s[:, :, 0], scalar=-2.0, in1=zsqT[:],
                    op0=mybir.AluOpType.mult, op1=mybir.AluOpType.add,
                )
                uT = small.tile([128, CH], f32, tag="u")
                nc.vector.tensor_tensor(out=uT[:], in0=wT[:],
                                        in1=sel[:, :, 0],
                                        op=mybir.AluOpType.add)
                f1T = small.tile([128, CH], f32, tag="f1")
                nc.vector.tensor_scalar(
                    out=f1T[:], in0=uT[:], scalar1=0.0, scalar2=None,
                    op0=mybir.AluOpType.is_gt,
                )
                f2T = small.tile([128, CH], f32, tag="f2")
                nc.vector.tensor_tensor(out=f2T[:], in0=wT[:],
                                        in1=sel[:, :, 1],
                                        op=mybir.AluOpType.is_lt)
                nc.vector.tensor_tensor(
                    out=driftacc[:, ip * CH:(ip + 1) * CH],
                    in0=f1T[:], in1=f2T[:], op=mybir.AluOpType.max,
                )

            pending = None

            for i in range(n_tiles):
                xnb = xin.tile([128, 2, 2, TOK_TILE], fp8, tag="xin")
                nc.sync.dma_start(xnb[:], xnT[i])

                if pending is not None:
                    emit_sel_flags(pending)
                    pending = None

                # ---- layer 1 ------------------------------------------
                z2 = z2bufs[i % 2]
                h2 = hsb.tile([128, 2, TOK_TILE], fp8, tag="h")
                if b1_zero:
                    hT = mmp.tile([128, 2, TOK_TILE], f32, tag="mm")
                    for fc in range(2):
                        for kc2 in range(2):
                            nc.tensor.matmul(
                                hT[:, fc, :],
                                lhsT=W1s[:, kc2, :, fc * 128:(fc + 1) * 128],
                                rhs=xnb[:, kc2],
                                start=(kc2 == 0), stop=(kc2 == 1),
                                perf_mode=DR,
                            )
                    nc.scalar.activation(
                        h2[:], hT[:], mybir.ActivationFunctionType.Relu,
                        scale=SC_H / (SC_X * SC_W1),
                    )
                else:
                    for fc in range(2):
                        hT = mmp.tile([128, 2, TOK_TILE], f32, tag="mm")
                        for kc2 in range(2):
                            nc.tensor.matmul(
                                hT[:, fc, :],
                                lhsT=W1s[:, kc2, :, fc * 128:(fc + 1) * 128],
                                rhs=xnb[:, kc2],
                                start=(kc2 == 0), stop=(kc2 == 1),
                                perf_mode=DR,
                            )
                        nc.scalar.activation(
                            h2[:, fc, :], hT[:, fc, :],
                            mybir.ActivationFunctionType.Relu,
                            bias=b1s[:, fc:fc + 1],
                            scale=SC_H / (SC_X * SC_W1),
                        )

                # ---- layer 2, feature-major ---------------------------
                zT = mmp.tile([128, 2, TOK_TILE], f32, tag="mm")
                nc.tensor.matmul(zT[:, 0, :], lhsT=W2s[:], rhs=h2[:],
                                 start=True, stop=True, perf_mode=DR)
                nc.scalar.activation(
                    z2[:, 0, :], zT[:, 0, :],
                    mybir.ActivationFunctionType.Copy,
                    scale=SC_Z / (SC_H * SC_W2),
                )

                # ---- layer 2, token-major (for ||z'||^2) --------------
                ztk = ztpp.tile([128, CH, D_LAT], f32, tag="ztk")
                for c in range(CH):
                    csl = slice(c * 128, (c + 1) * 128)
                    nc.tensor.matmul(
                        ztk[:, c, :], lhsT=h2[:, :, csl], rhs=W2s[:],
                        start=True, stop=True, perf_mode=DR,
                    )
                z2t = z2tp.tile([128, CH, D_LAT], bf16, tag="z2t")
                nc.scalar.activation(
                    z2t[:], ztk[:], mybir.ActivationFunctionType.Copy,
                    scale=SC_Z / (SC_H * SC_W2),
                )

                zsqT = tap.tile([128, CH], f32, tag="zsq")
                m8s = tap.tile([128, CH, 8], f32, tag="m8")
                ohTs = []

                for c in range(CH):
                    csl = slice(c * 128, (c + 1) * 128)

                    jz = junkp.tile([128, D_LAT], bf16, tag="jz")
                    nc.vector.scalar_tensor_tensor(
                        out=jz[:], in0=z2t[:, c, :], scalar=1.0,
                        in1=z2t[:, c, :],
                        op0=mybir.AluOpType.mult,
                        op1=mybir.AluOpType.mult,
                        accum_out=zsqT[:, c:c + 1],
                    )

                    # ---- scores ---------------------------------------
                    MP = gpp.tile([128, 1024], f32, tag="MP")
                    for lo, hi in halves:
                        nc.tensor.matmul(
                            MP[:, lo:lo + (hi - lo)],
                            lhsT=z2[:, :, csl], rhs=cMs[:, :, lo:hi],
                            start=True, stop=True, perf_mode=DR,
                        )

                    # PSUM -> SBUF bf16 (ACT x3 / DVE x1)
                    Msb = msbufs[(i * CH + c) % 3]
                    if c < 3:
                        nc.scalar.activation(
                            Msb[:, 0:K], MP[:, 0:K],
                            mybir.ActivationFunctionType.Copy,
                        )
                    else:
                        nc.vector.tensor_copy(out=Msb[:, 0:K],
                                              in_=MP[:, 0:K])

                    # ---- m1 via TT-max tree + InstMax -----------------
                    t5 = treep.tile([128, 500], bf16, tag="t5")
                    nc.vector.tensor_tensor(
                        out=t5[:], in0=Msb[:, 0:500], in1=Msb[:, 500:1000],
                        op=mybir.AluOpType.max,
                    )
                    t2 = treep.tile([128, 250], bf16, tag="t2")
                    nc.vector.tensor_tensor(
                        out=t2[:], in0=t5[:, 0:250], in1=t5[:, 250:500],
                        op=mybir.AluOpType.max,
                    )
                    nc.vector.max(out=m8s[:, c, :], in_=t2[:])

                    # ---- onehot (DVE 4x) + xbar transpose -------------
                    oh = ohp.tile([128, KP], bf16, tag="oh")
                    nc.vector.tensor_scalar(
                        out=oh[:], in0=Msb[:], scalar1=m8s[:, c, 0:1],
                        scalar2=None, op0=mybir.AluOpType.is_ge,
                    )
                    ohT = ohtp.tile([128, 8, 128], bf16, tag="ohT")
                    nc.sync.dma_start_transpose(ohT[:], oh[:])
                    ohTs.append(ohT)

                pending = (i, ohTs, m8s, zsqT)

            emit_sel_flags(pending)

            # ---- transpose to token order and store -----------------------
            ncols = n_tiles * CH
            tpsum = gpp.tile([128, 128], f32, tag="MP")
            nc.tensor.transpose(tpsum[:ncols, :], driftacc[:, :ncols],
                                ident[:])
            drift_i = small.tile([128, 128], i32, tag="drifti")
            nc.vector.tensor_copy(out=drift_i[:ncols, :], in_=tpsum[:ncols, :])
            nc.sync.dma_start(
                drift_d.rearrange("(a b) -> a b", b=128),
                drift_i[:ncols, :],
            )

    nc.compile()
    return nc


def prep_inputs(x, noise, W1, b1, W2, b2, centroid, dis_median, mad,
                n_tiles=BS // TOK_TILE, n_cores=N_CORES):
    bs = n_tiles * TOK_TILE
    x = np.asarray(x, dtype=np.float32)
    noise = np.asarray(noise, dtype=np.float32)
    W1 = np.asarray(W1, dtype=np.float32)
    b1 = np.asarray(b1, dtype=np.float32)
    W2 = np.asarray(W2, dtype=np.float32)
    b2 = np.asarray(b2, dtype=np.float32)
    centroid = np.asarray(centroid, dtype=np.float32)
    dis_median = np.asarray(dis_median, dtype=np.float32)
    mad = np.asarray(mad, dtype=np.float32)

    xn = x + noise

    W1s = np.ascontiguousarray(
        (W1 * SC_W1).reshape(2, 2, 128, H).transpose(2, 0, 1, 3)).astype(FP8)
    W2s = np.ascontiguousarray(
        (W2 * SC_W2).reshape(2, 128, D_LAT).transpose(1, 0, 2)).astype(FP8)
    b1s = np.ascontiguousarray((b1 * SC_H).reshape(2, 128).T)

    cp = centroid - b2[None, :]
    c2 = (cp * cp).sum(1)
    q = PRE_S - 0.5 * c2
    qhi = (q * SC_Z).astype(FP8)
    qlo = ((q - qhi.astype(np.float32) / SC_Z) * SC_Z).astype(FP8)

    cM = np.zeros((128, 2, K), dtype=FP8)
    cM[:, 0, :] = (cp.T * SC_Z).astype(FP8)
    cM[0, 1, :] = qhi
    cM[1, 1, :] = qlo

    hi = dis_median + MAD_THRESHOLD * mad
    lo = dis_median - MAD_THRESHOLD * mad
    A = (hi * hi).astype(np.float32)
    Bv = np.where(lo > 0, lo * lo, -1.0).astype(np.float32)
    E = 256.0 * (2.0 * PRE_S - A)
    Dv = 256.0 * Bv - 512.0 * PRE_S
    ED = np.zeros((128, 8, 2), dtype=BF16)
    j = np.arange(K)
    ED[j % 128, j // 128, 0] = E.astype(BF16)
    ED[j % 128, j // 128, 1] = Dv.astype(BF16)

    def shard_xn(core):
        s = (xn[core * bs:(core + 1) * bs] * SC_X).astype(FP8)
        a = s.T.reshape(2, 2, 128, n_tiles, TOK_TILE)
        return np.ascontiguousarray(a.transpose(3, 2, 0, 1, 4))

    in_maps = []
    for core in range(n_cores):
        in_maps.append({
            "xnT": shard_xn(core),
            "W1d": W1s,
            "W2d": W2s,
            "cMd": cM,
            "EDd": ED,
            "b1d": b1s,
        })
    return in_maps


_BUILD_CACHE = {}


def kernel(x, noise, W1, b1, W2, b2, centroid, dis_median, mad):
    from concourse.bass_utils import run_bass_kernel_spmd

    b1_zero = bool(np.all(np.asarray(b1) == 0))
    key = ("nc", b1_zero)
    nc = _BUILD_CACHE.get(key)
    if nc is None:
        nc = _BUILD_CACHE[key] = build_program(b1_zero=b1_zero)
    in_maps = prep_inputs(x, noise, W1, b1, W2, b2, centroid,
                          dis_median, mad)
    res = run_bass_kernel_spmd(nc, in_maps, core_ids=list(range(N_CORES)))
    out = np.concatenate([r["drift"] for r in res.results])
    return out.astype(np.int32)


# revision 11
# speedup vs baseline: 1.0051x; 1.0051x over previous
"""Trainium2 Bass kernel for nn_Detector (retrieval_knn drift detector).

Math (per token):
    z  = encoder(x + noise) = relu((x+n) @ W1 + b1) @ W2 + b2
    cls = argmin_j ||z - c_j||     (reference uses encoder(x); noise is 1e-2
                                    so computing the argmin on the noisy
                                    encoding leaves the drift bit unchanged)
    d2 = ||z - c_cls||^2
    drift = (d2 > A_cls) | (d2 < B_cls),  A=(med+3.5 mad)^2, B=(med-3.5 mad)^2

Host algebra (c' = c - b2, z' = bias-free encoding):
    argmin_j ||z - c_j|| == argmax_j M_j,  M_j = 256*(z'.c'_j + q_j),
        q_j = S - 0.5||c'_j||^2
    m1 = max_j M_j  =>  256*d2 = 256*||z'||^2 - 2*m1 + 512*S  (c'* cancels)
    onehot = (M >= m1)  -- DVE tensor_scalar at 4x -- is DMA-transposed
    (hw xbar) to [j-part, tok] layout, then a tiny PE matmul with the
    constant table [E_j | D_j] selects both thresholds exactly:
        E_j = 256*(2S - A_j),  D_j = 256*B_j - 512*S
        f1 = (w + E* > 0),  f2 = (w < D*),  w = 256*||z'||^2 - 2*m1
    drift = f1 | f2.

All big matmuls fp8e4 DoubleRow (K=256 packed / 128 partitions, 0.5
cycles/row); q rides in contraction rows 128/129 (hi+lo fp8 split).
Scales: x*16, W1*64, h*32, W2*32, z*16, c*16 => scores/d2 in 256*units.

The select matmuls + drift flags for tile i are emitted during tile i+1
(one-stage software pipeline) so the PE never stalls on the
copy->tree->onehot->transpose latency chain.

Pure data-parallel over 8 NeuronCores (8192 tokens each).
"""

import numpy as np
import ml_dtypes

import concourse.bass as bass
import concourse.bacc as bacc
import concourse.mybir as mybir
import concourse.tile as tile
from concourse.masks import make_identity

FP8 = ml_dtypes.float8_e4m3
BF16 = ml_dtypes.bfloat16

B, D_IN, H, D_LAT, K = 65536, 512, 256, 128, 1000
MAD_THRESHOLD = 3.5
N_CORES = 8
BS = B // N_CORES            # tokens per core
TOK_TILE = 512               # tokens per pipeline tile
CH = TOK_TILE // 128         # 4 token chunks per tile
KP = 1024                    # padded centroid count (xbar needs /128)
PRE_S = 64.0                 # q shift

SC_X = 16.0
SC_W1 = 64.0
SC_H = 32.0
SC_W2 = 32.0
SC_Z = 16.0

DR = mybir.MatmulPerfMode.DoubleRow


def build_program(n_tiles=BS // TOK_TILE, b1_zero=True):
    bs = n_tiles * TOK_TILE
    nc = bacc.Bacc(
        "TRN2",
        target_bir_lowering=False,
        debug=False,
        enable_asserts=False,
        num_devices=N_CORES,
    )
    f32, bf16, fp8, i32 = (
        mybir.dt.float32, mybir.dt.bfloat16, mybir.dt.float8e4, mybir.dt.int32,
    )

    xnT = nc.dram_tensor("xnT", [n_tiles, 128, 2, 2, TOK_TILE], fp8,
                         kind="ExternalInput").ap()
    W1d = nc.dram_tensor("W1d", [128, 2, 2, H], fp8, kind="ExternalInput").ap()
    W2d = nc.dram_tensor("W2d", [128, 2, D_LAT], fp8,
                         kind="ExternalInput").ap()
    cMd = nc.dram_tensor("cMd", [128, 2, K], fp8, kind="ExternalInput").ap()
    EDd = nc.dram_tensor("EDd", [128, 8, 2], bf16, kind="ExternalInput").ap()
    b1d = nc.dram_tensor("b1d", [128, 2], f32, kind="ExternalInput").ap()
    drift_d = nc.dram_tensor("drift", [bs], i32, kind="ExternalOutput").ap()

    with tile.TileContext(nc) as tc:
        with (
            tc.tile_pool(name="const", bufs=1) as const,
            tc.tile_pool(name="xin", bufs=3) as xin,
            tc.tile_pool(name="hsb", bufs=2) as hsb,
            tc.tile_pool(name="z2tp", bufs=2) as z2tp,
            tc.tile_pool(name="tree", bufs=3) as treep,
            tc.tile_pool(name="ohp", bufs=3) as ohp,
            tc.tile_pool(name="ohtp", bufs=8) as ohtp,
            tc.tile_pool(name="junk", bufs=3) as junkp,
            tc.tile_pool(name="ta", bufs=6) as tap,
            tc.tile_pool(name="flags", bufs=10) as small,
            tc.tile_pool(name="acc", bufs=1) as accp,
            tc.tile_pool(name="mm", bufs=1, space="PSUM") as mmp,
            tc.tile_pool(name="ztp", bufs=1, space="PSUM") as ztpp,
            tc.tile_pool(name="selp", bufs=1, space="PSUM") as selpp,
            tc.tile_pool(name="gp", bufs=2, space="PSUM") as gpp,
        ):
            # ---- constants -------------------------------------------------
            W1s = const.tile([128, 2, 2, H], fp8)
            nc.sync.dma_start(W1s[:], W1d[:])
            W2s = const.tile([128, 2, D_LAT], fp8)
            nc.sync.dma_start(W2s[:], W2d[:])
            cMs = const.tile([128, 2, K], fp8)
            nc.sync.dma_start(cMs[:], cMd[:])
            EDs = const.tile([128, 8, 2], bf16)
            nc.sync.dma_start(EDs[:], EDd[:])
            b1s = const.tile([128, 2], f32)
            nc.sync.dma_start(b1s[:], b1d[:])
            ident = const.tile([128, 128], f32)
            make_identity(nc, ident[:])

            msbufs = []
            for k in range(4):
                mb = const.tile([128, KP], bf16, tag=f"mb{k}")
                nc.gpsimd.memset(mb[:, K:KP], 0.0)
                msbufs.append(mb)

            z2bufs = []
            for k in range(2):
                zb = const.tile([128, 2, TOK_TILE], fp8, tag=f"zb{k}")
                nc.gpsimd.memset(zb[:, 1, :], 0.0)
                nc.gpsimd.memset(zb[0:2, 1, :], SC_Z)
                z2bufs.append(zb)

            driftacc = accp.tile([128, n_tiles * CH], f32)

            halves = [(0, 512), (512, K)]

            def emit_sel_flags(st):
                """Select matmuls + drift flags for a finished tile."""
                ip, ohTs, m8s, zsqT = st
                sel = selpp.tile([128, CH, 2], f32, tag="sel")
                for c in range(CH):
                    for g in range(8):
                        nc.tensor.matmul(
                            sel[:, c, :], lhsT=ohTs[c][:, g, :],
                            rhs=EDs[:, g, :],
                            start=(g == 0), stop=(g == 7),
                        )
                wT = small.tile([128, CH], f32, tag="w")
                nc.vector.scalar_tensor_tensor(
                    out=wT[:], in0=m8s[:, :, 0], scalar=-2.0, in1=zsqT[:],
                    op0=mybir.AluOpType.mult, op1=mybir.AluOpType.add,
                )
                uT = small.tile([128, CH], f32, tag="u")
                nc.vector.tensor_tensor(out=uT[:], in0=wT[:],
                                        in1=sel[:, :, 0],
                                        op=mybir.AluOpType.add)
                f1T = small.tile([128, CH], f32, tag="f1")
                nc.vector.tensor_scalar(
                    out=f1T[:], in0=uT[:], scalar1=0.0, scalar2=None,
                    op0=mybir.AluOpType.is_gt,
                )
                f2T = small.tile([128, CH], f32, tag="f2")
                nc.vector.tensor_tensor(out=f2T[:], in0=wT[:],
                                        in1=sel[:, :, 1],
                                        op=mybir.AluOpType.is_lt)
                nc.vector.tensor_tensor(
                    out=driftacc[:, ip * CH:(ip + 1) * CH],
                    in0=f1T[:], in1=f2T[:], op=mybir.AluOpType.max,
                )

            pending = None

            for i in range(n_tiles):
                xnb = xin.tile([128, 2, 2, TOK_TILE], fp8, tag="xin")
                nc.sync.dma_start(xnb[:], xnT[i])

                if pending is not None:
                    emit_sel_flags(pending)
                    pending = None

                # ---- layer 1 ------------------------------------------
                z2 = z2bufs[i % 2]
                h2 = hsb.tile([128, 2, TOK_TILE], fp8, tag="h")
                if b1_zero:
                    hT = mmp.tile([128, 2, TOK_TILE], f32, tag="mm")
                    for fc in range(2):
                        for kc2 in range(2):
                            nc.tensor.matmul(
                                hT[:, fc, :],
                                lhsT=W1s[:, kc2, :, fc * 128:(fc + 1) * 128],
                                rhs=xnb[:, kc2],
                                start=(kc2 == 0), stop=(kc2 == 1),
                                perf_mode=DR,
                            )
                    nc.scalar.activation(
                        h2[:], hT[:], mybir.ActivationFunctionType.Relu,
                        scale=SC_H / (SC_X * SC_W1),
                    )
                else:
                    for fc in range(2):
                        hT = mmp.tile([128, 2, TOK_TILE], f32, tag="mm")
                        for kc2 in range(2):
                            nc.tensor.matmul(
                                hT[:, fc, :],
                                lhsT=W1s[:, kc2, :, fc * 128:(fc + 1) * 128],
                                rhs=xnb[:, kc2],
                                start=(kc2 == 0), stop=(kc2 == 1),
                                perf_mode=DR,
                            )
                        nc.scalar.activation(
                            h2[:, fc, :], hT[:, fc, :],
                            mybir.ActivationFunctionType.Relu,
                            bias=b1s[:, fc:fc + 1],
                            scale=SC_H / (SC_X * SC_W1),
                        )

                # ---- layer 2, feature-major ---------------------------
                zT = mmp.tile([128, 2, TOK_TILE], f32, tag="mm")
                nc.tensor.matmul(zT[:, 0, :], lhsT=W2s[:], rhs=h2[:],
                                 start=True, stop=True, perf_mode=DR)
                nc.scalar.activation(
                    z2[:, 0, :], zT[:, 0, :],
                    mybir.ActivationFunctionType.Copy,
                    scale=SC_Z / (SC_H * SC_W2),
                )

                # ---- layer 2, token-major (for ||z'||^2) --------------
                ztk = ztpp.tile([128, CH, D_LAT], f32, tag="ztk")
                for c in range(CH):
                    csl = slice(c * 128, (c + 1) * 128)
                    nc.tensor.matmul(
                        ztk[:, c, :], lhsT=h2[:, :, csl], rhs=W2s[:],
                        start=True, stop=True, perf_mode=DR,
                    )
                z2t = z2tp.tile([128, CH, D_LAT], bf16, tag="z2t")
                nc.scalar.activation(
                    z2t[:], ztk[:], mybir.ActivationFunctionType.Copy,
                    scale=SC_Z / (SC_H * SC_W2),
                )

                zsqT = tap.tile([128, CH], f32, tag="zsq")
                m8s = tap.tile([128, CH, 8], f32, tag="m8")
                ohTs = []

                for c in range(CH):
                    csl = slice(c * 128, (c + 1) * 128)

                    jz = junkp.tile([128, D_LAT], bf16, tag="jz")
                    nc.vector.scalar_tensor_tensor(
                        out=jz[:], in0=z2t[:, c, :], scalar=1.0,
                        in1=z2t[:, c, :],
                        op0=mybir.AluOpType.mult,
                        op1=mybir.AluOpType.mult,
                        accum_out=zsqT[:, c:c + 1],
                    )

                    # ---- scores ---------------------------------------
                    MP = gpp.tile([128, 1024], f32, tag="MP")
                    for lo, hi in halves:
                        nc.tensor.matmul(
                            MP[:, lo:lo + (hi - lo)],
                            lhsT=z2[:, :, csl], rhs=cMs[:, :, lo:hi],
                            start=True, stop=True, perf_mode=DR,
                        )

                    # PSUM -> SBUF bf16 (ACT x3 / DVE x1)
                    Msb = msbufs[c]
                    if c < 3:
                        nc.scalar.activation(
                            Msb[:, 0:K], MP[:, 0:K],
                            mybir.ActivationFunctionType.Copy,
                        )
                    else:
                        nc.vector.tensor_copy(out=Msb[:, 0:K],
                                              in_=MP[:, 0:K])

                    # ---- m1 via TT-max tree + InstMax -----------------
                    t5 = treep.tile([128, 500], bf16, tag="t5")
                    nc.vector.tensor_tensor(
                        out=t5[:], in0=Msb[:, 0:500], in1=Msb[:, 500:1000],
                        op=mybir.AluOpType.max,
                    )
                    t2 = treep.tile([128, 250], bf16, tag="t2")
                    nc.vector.tensor_tensor(
                        out=t2[:], in0=t5[:, 0:250], in1=t5[:, 250:500],
                        op=mybir.AluOpType.max,
                    )
                    nc.vector.max(out=m8s[:, c, :], in_=t2[:])

                    # ---- onehot (DVE 4x) + xbar transpose -------------
                    oh = ohp.tile([128, KP], bf16, tag="oh")
                    nc.vector.tensor_scalar(
                        out=oh[:], in0=Msb[:], scalar1=m8s[:, c, 0:1],
                        scalar2=None, op0=mybir.AluOpType.is_ge,
                    )
                    ohT = ohtp.tile([128, 8, 128], bf16, tag="ohT")
                    nc.sync.dma_start_transpose(ohT[:], oh[:])
                    ohTs.append(ohT)

                pending = (i, ohTs, m8s, zsqT)

            emit_sel_flags(pending)

            # ---- transpose to token order and store -----------------------
            ncols = n_tiles * CH
            tpsum = gpp.tile([128, 128], f32, tag="MP")
            nc.tensor.transpose(tpsum[:ncols, :], driftacc[:, :ncols],
                                ident[:])
            drift_i = small.tile([128, 128], i32, tag="drifti")
            nc.vector.tensor_copy(out=drift_i[:ncols, :], in_=tpsum[:ncols, :])
            nc.sync.dma_start(
                drift_d.rearrange("(a b) -> a b", b=128),
                drift_i[:ncols, :],
            )

    nc.compile()
    return nc


def prep_inputs(x, noise, W1, b1, W2, b2, centroid, dis_median, mad,
                n_tiles=BS // TOK_TILE, n_cores=N_CORES):
    bs = n_tiles * TOK_TILE
    x = np.asarray(x, dtype=np.float32)
    noise = np.asarray(noise, dtype=np.float32)
    W1 = np.asarray(W1, dtype=np.float32)
    b1 = np.asarray(b1, dtype=np.float32)
    W2 = np.asarray(W2, dtype=np.float32)
    b2 = np.asarray(b2, dtype=np.float32)
    centroid = np.asarray(centroid, dtype=np.float32)
    dis_median = np.asarray(dis_median, dtype=np.float32)
    mad = np.asarray(mad, dtype=np.float32)

    xn = x + noise

    W1s = np.ascontiguousarray(
        (W1 * SC_W1).reshape(2, 2, 128, H).transpose(2, 0, 1, 3)).astype(FP8)
    W2s = np.ascontiguousarray(
        (W2 * SC_W2).reshape(2, 128, D_LAT).transpose(1, 0, 2)).astype(FP8)
    b1s = np.ascontiguousarray((b1 * SC_H).reshape(2, 128).T)

    cp = centroid - b2[None, :]
    c2 = (cp * cp).sum(1)
    q = PRE_S - 0.5 * c2
    qhi = (q * SC_Z).astype(FP8)
    qlo = ((q - qhi.astype(np.float32) / SC_Z) * SC_Z).astype(FP8)

    cM = np.zeros((128, 2, K), dtype=FP8)
    cM[:, 0, :] = (cp.T * SC_Z).astype(FP8)
    cM[0, 1, :] = qhi
    cM[1, 1, :] = qlo

    hi = dis_median + MAD_THRESHOLD * mad
    lo = dis_median - MAD_THRESHOLD * mad
    A = (hi * hi).astype(np.float32)
    Bv = np.where(lo > 0, lo * lo, -1.0).astype(np.float32)
    E = 256.0 * (2.0 * PRE_S - A)
    Dv = 256.0 * Bv - 512.0 * PRE_S
    ED = np.zeros((128, 8, 2), dtype=BF16)
    j = np.arange(K)
    ED[j % 128, j // 128, 0] = E.astype(BF16)
    ED[j % 128, j // 128, 1] = Dv.astype(BF16)

    def shard_xn(core):
        s = (xn[core * bs:(core + 1) * bs] * SC_X).astype(FP8)
        a = s.T.reshape(2, 2, 128, n_tiles, TOK_TILE)
        return np.ascontiguousarray(a.transpose(3, 2, 0, 1, 4))

    in_maps = []
    for core in range(n_cores):
        in_maps.append({
            "xnT": shard_xn(core),
            "W1d": W1s,
            "W2d": W2s,
            "cMd": cM,
            "EDd": ED,
            "b1d": b1s,
        })
    return in_maps


_BUILD_CACHE = {}


def kernel(x, noise, W1, b1, W2, b2, centroid, dis_median, mad):
    from concourse.bass_utils import run_bass_kernel_spmd

    b1_zero = bool(np.all(np.asarray(b1) == 0))
    key = ("nc", b1_zero)
    nc = _BUILD_CACHE.get(key)
    if nc is None:
        nc = _BUILD_CACHE[key] = build_program(b1_zero=b1_zero)
    in_maps = prep_inputs(x, noise, W1, b1, W2, b2, centroid,
                          dis_median, mad)
    res = run_bass_kernel_spmd(nc, in_maps, core_ids=list(range(N_CORES)))
    out = np.concatenate([r["drift"] for r in res.results])
    return out.astype(np.int32)


# revision 13
# speedup vs baseline: 1.0341x; 1.0289x over previous
"""Trainium2 Bass kernel for nn_Detector (retrieval_knn drift detector).

Math (per token):
    z  = encoder(x + noise) = relu((x+n) @ W1 + b1) @ W2 + b2
    cls = argmin_j ||z - c_j||     (reference uses encoder(x); noise is 1e-2
                                    so computing the argmin on the noisy
                                    encoding leaves the drift bit unchanged)
    d2 = ||z - c_cls||^2
    drift = (d2 > A_cls) | (d2 < B_cls),  A=(med+3.5 mad)^2, B=(med-3.5 mad)^2

Host algebra (c' = c - b2, z' = bias-free encoding):
    argmin_j ||z - c_j|| == argmax_j M_j,  M_j = 256*(z'.c'_j + q_j),
        q_j = S - 0.5||c'_j||^2
    m1 = max_j M_j  =>  256*d2 = 256*||z'||^2 - 2*m1 + 512*S  (c'* cancels)
    onehot = (M >= m1)  -- DVE tensor_scalar at 4x -- is DMA-transposed
    (hw xbar) to [j-part, tok] layout, then a tiny PE matmul with the
    constant table [E_j | D_j] selects both thresholds exactly:
        E_j = 256*(2S - A_j),  D_j = 256*B_j - 512*S
        f1 = (w + E* > 0),  f2 = (w < D*),  w = 256*||z'||^2 - 2*m1
    drift = f1 | f2.

All big matmuls fp8e4 DoubleRow (K=256 packed / 128 partitions, 0.5
cycles/row); q rides in contraction rows 128/129 (hi+lo fp8 split).
Scales: x*16, W1*64, h*32, W2*32, z*16, c*16 => scores/d2 in 256*units.

The select matmuls + drift flags for tile i are emitted during tile i+1
(one-stage software pipeline) so the PE never stalls on the
copy->tree->onehot->transpose latency chain.

Pure data-parallel over 8 NeuronCores (8192 tokens each).
"""

import numpy as np
import ml_dtypes

import concourse.bass as bass
import concourse.bacc as bacc
import concourse.mybir as mybir
import concourse.tile as tile
from concourse.masks import make_identity

FP8 = ml_dtypes.float8_e4m3
BF16 = ml_dtypes.bfloat16

B, D_IN, H, D_LAT, K = 65536, 512, 256, 128, 1000
MAD_THRESHOLD = 3.5
N_CORES = 8
BS = B // N_CORES            # tokens per core
TOK_TILE = 512               # tokens per pipeline tile
CH = TOK_TILE // 128         # 4 token chunks per tile
KP = 1024                    # padded centroid count (xbar needs /128)
PRE_S = 64.0                 # q shift

SC_X = 16.0
SC_W1 = 64.0
SC_H = 32.0
SC_W2 = 32.0
SC_Z = 16.0

DR = mybir.MatmulPerfMode.DoubleRow


def build_program(n_tiles=BS // TOK_TILE, b1_zero=True):
    bs = n_tiles * TOK_TILE
    nc = bacc.Bacc(
        "TRN2",
        target_bir_lowering=False,
        debug=False,
        enable_asserts=False,
        num_devices=N_CORES,
    )
    f32, bf16, fp8, i32 = (
        mybir.dt.float32, mybir.dt.bfloat16, mybir.dt.float8e4, mybir.dt.int32,
    )

    xnT = nc.dram_tensor("xnT", [n_tiles, 128, 2, 2, TOK_TILE], fp8,
                         kind="ExternalInput").ap()
    W1d = nc.dram_tensor("W1d", [128, 2, 2, H], fp8, kind="ExternalInput").ap()
    W2d = nc.dram_tensor("W2d", [128, 2, D_LAT], fp8,
                         kind="ExternalInput").ap()
    cMd = nc.dram_tensor("cMd", [128, 2, K], fp8, kind="ExternalInput").ap()
    EDd = nc.dram_tensor("EDd", [128, 8, 2], bf16, kind="ExternalInput").ap()
    b1d = nc.dram_tensor("b1d", [128, 2], f32, kind="ExternalInput").ap()
    drift_d = nc.dram_tensor("drift", [bs], i32, kind="ExternalOutput").ap()

    with tile.TileContext(nc) as tc:
        with (
            tc.tile_pool(name="const", bufs=1) as const,
            tc.tile_pool(name="xin", bufs=3) as xin,
            tc.tile_pool(name="hsb", bufs=2) as hsb,
            tc.tile_pool(name="z2tp", bufs=2) as z2tp,
            tc.tile_pool(name="tree", bufs=4) as treep,
            tc.tile_pool(name="ohp", bufs=4) as ohp,
            tc.tile_pool(name="ohtp", bufs=8) as ohtp,
            tc.tile_pool(name="junk", bufs=3) as junkp,
            tc.tile_pool(name="ta", bufs=6) as tap,
            tc.tile_pool(name="flags", bufs=10) as small,
            tc.tile_pool(name="acc", bufs=1) as accp,
            tc.tile_pool(name="mm", bufs=1, space="PSUM") as mmp,
            tc.tile_pool(name="ztp", bufs=1, space="PSUM") as ztpp,
            tc.tile_pool(name="selp", bufs=1, space="PSUM") as selpp,
            tc.tile_pool(name="gp", bufs=2, space="PSUM") as gpp,
        ):
            # ---- constants -------------------------------------------------
            W1s = const.tile([128, 2, 2, H], fp8)
            nc.sync.dma_start(W1s[:], W1d[:])
            W2s = const.tile([128, 2, D_LAT], fp8)
            nc.sync.dma_start(W2s[:], W2d[:])
            cMs = const.tile([128, 2, K], fp8)
            nc.sync.dma_start(cMs[:], cMd[:])
            EDs = const.tile([128, 8, 2], bf16)
            nc.sync.dma_start(EDs[:], EDd[:])
            b1s = const.tile([128, 2], f32)
            nc.sync.dma_start(b1s[:], b1d[:])
            ident = const.tile([128, 128], f32)
            make_identity(nc, ident[:])

            msbufs = []
            for k in range(4):
                mb = const.tile([128, KP], bf16, tag=f"mb{k}")
                nc.gpsimd.memset(mb[:, K:KP], 0.0)
                msbufs.append(mb)

            z2bufs = []
            for k in range(2):
                zb = const.tile([128, 2, TOK_TILE], fp8, tag=f"zb{k}")
                nc.gpsimd.memset(zb[:, 1, :], 0.0)
                nc.gpsimd.memset(zb[0:2, 1, :], SC_Z)
                z2bufs.append(zb)

            driftacc = accp.tile([128, n_tiles * CH], f32)

            halves = [(0, 512), (512, K)]

            def emit_sel_flags(st):
                """Select matmuls + drift flags for a finished tile."""
                ip, ohTs, m8s, zsqT = st
                sel = selpp.tile([128, CH, 2], f32, tag="sel")
                for c in range(CH):
                    for g in range(8):
                        nc.tensor.matmul(
                            sel[:, c, :], lhsT=ohTs[c][:, g, :],
                            rhs=EDs[:, g, :],
                            start=(g == 0), stop=(g == 7),
                        )
                wT = small.tile([128, CH], f32, tag="w")
                nc.vector.scalar_tensor_tensor(
                    out=wT[:], in0=m8s[:, :, 0], scalar=-2.0, in1=zsqT[:],
                    op0=mybir.AluOpType.mult, op1=mybir.AluOpType.add,
                )
                uT = small.tile([128, CH], f32, tag="u")
                nc.vector.tensor_tensor(out=uT[:], in0=wT[:],
                                        in1=sel[:, :, 0],
                                        op=mybir.AluOpType.add)
                f1T = small.tile([128, CH], f32, tag="f1")
                nc.vector.tensor_scalar(
                    out=f1T[:], in0=uT[:], scalar1=0.0, scalar2=None,
                    op0=mybir.AluOpType.is_gt,
                )
                f2T = small.tile([128, CH], f32, tag="f2")
                nc.vector.tensor_tensor(out=f2T[:], in0=wT[:],
                                        in1=sel[:, :, 1],
                                        op=mybir.AluOpType.is_lt)
                nc.vector.tensor_tensor(
                    out=driftacc[:, ip * CH:(ip + 1) * CH],
                    in0=f1T[:], in1=f2T[:], op=mybir.AluOpType.max,
                )

            pending = None

            for i in range(n_tiles):
                xnb = xin.tile([128, 2, 2, TOK_TILE], fp8, tag="xin")
                nc.sync.dma_start(xnb[:], xnT[i])

                if pending is not None:
                    emit_sel_flags(pending)
                    pending = None

                # ---- layer 1 ------------------------------------------
                z2 = z2bufs[i % 2]
                h2 = hsb.tile([128, 2, TOK_TILE], fp8, tag="h")
                if b1_zero:
                    hT = mmp.tile([128, 2, TOK_TILE], f32, tag="mm")
                    for fc in range(2):
                        for kc2 in range(2):
                            nc.tensor.matmul(
                                hT[:, fc, :],
                                lhsT=W1s[:, kc2, :, fc * 128:(fc + 1) * 128],
                                rhs=xnb[:, kc2],
                                start=(kc2 == 0), stop=(kc2 == 1),
                                perf_mode=DR,
                            )
                    nc.scalar.activation(
                        h2[:], hT[:], mybir.ActivationFunctionType.Relu,
                        scale=SC_H / (SC_X * SC_W1),
                    )
                else:
                    for fc in range(2):
                        hT = mmp.tile([128, 2, TOK_TILE], f32, tag="mm")
                        for kc2 in range(2):
                            nc.tensor.matmul(
                                hT[:, fc, :],
                                lhsT=W1s[:, kc2, :, fc * 128:(fc + 1) * 128],
                                rhs=xnb[:, kc2],
                                start=(kc2 == 0), stop=(kc2 == 1),
                                perf_mode=DR,
                            )
                        nc.scalar.activation(
                            h2[:, fc, :], hT[:, fc, :],
                            mybir.ActivationFunctionType.Relu,
                            bias=b1s[:, fc:fc + 1],
                            scale=SC_H / (SC_X * SC_W1),
                        )

                # ---- layer 2, feature-major ---------------------------
                zT = mmp.tile([128, 2, TOK_TILE], f32, tag="mm")
                nc.tensor.matmul(zT[:, 0, :], lhsT=W2s[:], rhs=h2[:],
                                 start=True, stop=True, perf_mode=DR)
                nc.scalar.activation(
                    z2[:, 0, :], zT[:, 0, :],
                    mybir.ActivationFunctionType.Copy,
                    scale=SC_Z / (SC_H * SC_W2),
                )

                # ---- layer 2, token-major (for ||z'||^2) --------------
                ztk = ztpp.tile([128, CH, D_LAT], f32, tag="ztk")
                for c in range(CH):
                    csl = slice(c * 128, (c + 1) * 128)
                    nc.tensor.matmul(
                        ztk[:, c, :], lhsT=h2[:, :, csl], rhs=W2s[:],
                        start=True, stop=True, perf_mode=DR,
                    )
                z2t = z2tp.tile([128, CH, D_LAT], bf16, tag="z2t")
                nc.scalar.activation(
                    z2t[:], ztk[:], mybir.ActivationFunctionType.Copy,
                    scale=SC_Z / (SC_H * SC_W2),
                )

                zsqT = tap.tile([128, CH], f32, tag="zsq")
                m8s = tap.tile([128, CH, 8], f32, tag="m8")
                ohTs = []

                # Stage: score matmuls + PSUM->SBUF copies, interleaved so
                # the 2-deep MP pool never stalls PE behind an unissued copy.
                # Chunk 0's copy goes to the DVE (its scores finish first).
                MPs = []
                copies = []

                def emit_mp(c):
                    csl = slice(c * 128, (c + 1) * 128)
                    MP = gpp.tile([128, 1024], f32, tag="MP")
                    for lo, hi in halves:
                        nc.tensor.matmul(
                            MP[:, lo:lo + (hi - lo)],
                            lhsT=z2[:, :, csl], rhs=cMs[:, :, lo:hi],
                            start=True, stop=True, perf_mode=DR,
                        )
                    MPs.append(MP)

                def emit_copy(c):
                    Msb = msbufs[c]
                    if c == 0:
                        nc.vector.tensor_copy(out=Msb[:, 0:K],
                                              in_=MPs[c][:, 0:K])
                    else:
                        nc.scalar.activation(
                            Msb[:, 0:K], MPs[c][:, 0:K],
                            mybir.ActivationFunctionType.Copy,
                        )

                emit_mp(0)
                emit_mp(1)
                emit_copy(0)
                emit_mp(2)
                emit_copy(1)
                emit_mp(3)
                emit_copy(2)
                emit_copy(3)

                # Stage: tree maxes (DVE) back to back
                for c in range(CH):
                    Msb = msbufs[c]
                    t5 = treep.tile([128, 500], bf16, tag="t5")
                    nc.vector.tensor_tensor(
                        out=t5[:], in0=Msb[:, 0:500], in1=Msb[:, 500:1000],
                        op=mybir.AluOpType.max,
                    )
                    t2 = treep.tile([128, 250], bf16, tag="t2")
                    nc.vector.tensor_tensor(
                        out=t2[:], in0=t5[:, 0:250], in1=t5[:, 250:500],
                        op=mybir.AluOpType.max,
                    )
                    nc.vector.max(out=m8s[:, c, :], in_=t2[:])

                    oh = ohp.tile([128, KP], bf16, tag="oh")
                    nc.vector.tensor_scalar(
                        out=oh[:], in0=Msb[:], scalar1=m8s[:, c, 0:1],
                        scalar2=None, op0=mybir.AluOpType.is_ge,
                    )
                    ohT = ohtp.tile([128, 8, 128], bf16, tag="ohT")
                    nc.sync.dma_start_transpose(ohT[:], oh[:])
                    ohTs.append(ohT)

                # Stage: zsq accumulations (feed next-tile flags)
                for c in range(CH):
                    jz = junkp.tile([128, D_LAT], bf16, tag="jz")
                    nc.vector.scalar_tensor_tensor(
                        out=jz[:], in0=z2t[:, c, :], scalar=1.0,
                        in1=z2t[:, c, :],
                        op0=mybir.AluOpType.mult,
                        op1=mybir.AluOpType.mult,
                        accum_out=zsqT[:, c:c + 1],
                    )

                pending = (i, ohTs, m8s, zsqT)

            emit_sel_flags(pending)

            # ---- transpose to token order and store -----------------------
            ncols = n_tiles * CH
            tpsum = gpp.tile([128, 128], f32, tag="MP")
            nc.tensor.transpose(tpsum[:ncols, :], driftacc[:, :ncols],
                                ident[:])
            drift_i = small.tile([128, 128], i32, tag="drifti")
            nc.vector.tensor_copy(out=drift_i[:ncols, :], in_=tpsum[:ncols, :])
            nc.sync.dma_start(
                drift_d.rearrange("(a b) -> a b", b=128),
                drift_i[:ncols, :],
            )

    nc.compile()
    return nc


def prep_inputs(x, noise, W1, b1, W2, b2, centroid, dis_median, mad,
                n_tiles=BS // TOK_TILE, n_cores=N_CORES):
    bs = n_tiles * TOK_TILE
    x = np.asarray(x, dtype=np.float32)
    noise = np.asarray(noise, dtype=np.float32)
    W1 = np.asarray(W1, dtype=np.float32)
    b1 = np.asarray(b1, dtype=np.float32)
    W2 = np.asarray(W2, dtype=np.float32)
    b2 = np.asarray(b2, dtype=np.float32)
    centroid = np.asarray(centroid, dtype=np.float32)
    dis_median = np.asarray(dis_median, dtype=np.float32)
    mad = np.asarray(mad, dtype=np.float32)

    xn = x + noise

    W1s = np.ascontiguousarray(
        (W1 * SC_W1).reshape(2, 2, 128, H).transpose(2, 0, 1, 3)).astype(FP8)
    W2s = np.ascontiguousarray(
        (W2 * SC_W2).reshape(2, 128, D_LAT).transpose(1, 0, 2)).astype(FP8)
    b1s = np.ascontiguousarray((b1 * SC_H).reshape(2, 128).T)

    cp = centroid - b2[None, :]
    c2 = (cp * cp).sum(1)
    q = PRE_S - 0.5 * c2
    qhi = (q * SC_Z).astype(FP8)
    qlo = ((q - qhi.astype(np.float32) / SC_Z) * SC_Z).astype(FP8)

    cM = np.zeros((128, 2, K), dtype=FP8)
    cM[:, 0, :] = (cp.T * SC_Z).astype(FP8)
    cM[0, 1, :] = qhi
    cM[1, 1, :] = qlo

    hi = dis_median + MAD_THRESHOLD * mad
    lo = dis_median - MAD_THRESHOLD * mad
    A = (hi * hi).astype(np.float32)
    Bv = np.where(lo > 0, lo * lo, -1.0).astype(np.float32)
    E = 256.0 * (2.0 * PRE_S - A)
    Dv = 256.0 * Bv - 512.0 * PRE_S
    ED = np.zeros((128, 8, 2), dtype=BF16)
    j = np.arange(K)
    ED[j % 128, j // 128, 0] = E.astype(BF16)
    ED[j % 128, j // 128, 1] = Dv.astype(BF16)

    def shard_xn(core):
        s = (xn[core * bs:(core + 1) * bs] * SC_X).astype(FP8)
        a = s.T.reshape(2, 2, 128, n_tiles, TOK_TILE)
        return np.ascontiguousarray(a.transpose(3, 2, 0, 1, 4))

    in_maps = []
    for core in range(n_cores):
        in_maps.append({
            "xnT": shard_xn(core),
            "W1d": W1s,
            "W2d": W2s,
            "cMd": cM,
            "EDd": ED,
            "b1d": b1s,
        })
    return in_maps


_BUILD_CACHE = {}


def kernel(x, noise, W1, b1, W2, b2, centroid, dis_median, mad):
    from concourse.bass_utils import run_bass_kernel_spmd

    b1_zero = bool(np.all(np.asarray(b1) == 0))
    key = ("nc", b1_zero)
    nc = _BUILD_CACHE.get(key)
    if nc is None:
        nc = _BUILD_CACHE[key] = build_program(b1_zero=b1_zero)
    in_maps = prep_inputs(x, noise, W1, b1, W2, b2, centroid,
                          dis_median, mad)
    res = run_bass_kernel_spmd(nc, in_maps, core_ids=list(range(N_CORES)))
    out = np.concatenate([r["drift"] for r in res.results])
    return out.astype(np.int32)
